# revision 13
# baseline (speedup 1.0000x reference)
# Laplacian normalization kernel for Trainium2 (8 NeuronCores, SPMD).
#
# out = d^-1/2[:, None] * A * d^-1/2[None, :],  d_i = sum_j A[i, j],  A: [8192, 8192] f32
#
# Sharding: row-wise across 8 cores (1024 rows each). Row sums are local; the
# column-scale vector needs the full d^-1/2 [8192], obtained with a tiny
# AllGather (4KB per core). Two passes over the shard per core:
#   pass 1: row sums in uniform small chunks (so the in-order DVE queue never
#           head-of-line blocks DMA slot recycling).
#   middle: rsqrt on [128, 8] (ACT sqrt + DVE reciprocal), PE-transpose to
#           [8, 128] so the collective input is written with ONE contiguous
#           4KB DMA (a [128,1]-per-tile scatter fragments into 4-byte DMA
#           descriptors), AllGather, then broadcast the gathered vector
#           across partitions in 4 chunked DMAs so pass-2 compute on chunk c
#           only waits for broadcast chunk c.
#   pass 2: out = (A * r_row) * c_col in one fused DVE op per chunk
#           (scalar_tensor_tensor), store per chunk.
#
# Queue discipline: ALL loads go on the Sync HWDGE queue; the broadcast and
# ALL stores go on the Scalar HWDGE queue. HWDGE queues execute in order, so
# putting the (collective-gated) broadcast on the load queue would block
# pass-2 prefetch from filling the otherwise-dead DMA window during the
# collective rendezvous.
#
# The first NCACHE row-tiles stay resident in SBUF between the passes (their
# pass-2 reload is free); the rest re-stream through 5 rotating 1MB chunk
# slots, which double as prefetch buffers during the collective window.
#
# SBUF/partition: 4*32KB cached + 5*8KB stream + 32KB cvec + ~1KB small
# = ~201KB of the ~208KB Tile exposes.

import numpy as np

N = 8192
NCORES = 8
R = N // NCORES  # 1024 rows per core
P = 128          # SBUF partitions
T = R // P       # 8 row-tiles of [128, 8192] per core
NCACHE = 4       # row-tiles kept resident in SBUF between passes
NCHUNK = 4       # column chunks per streamed row-tile (1MB each)
H = N // NCHUNK  # stream chunk width (2048 columns)
CCH = 2          # column chunks per cached row-tile (2MB each)
CH = N // CCH    # cached chunk width (4096 columns)

_cache = {}


def _build():
    import concourse.bacc as bacc
    import concourse.mybir as mybir
    import concourse.tile as tile
    from concourse import masks

    f32 = mybir.dt.float32
    X = mybir.AxisListType.X
    mult = mybir.AluOpType.mult

    nc = bacc.Bacc(
        "TRN2", target_bir_lowering=False, debug=False, num_devices=NCORES
    )
    a = nc.dram_tensor("a_shard", [R, N], f32, kind="ExternalInput").ap()
    out = nc.dram_tensor("out_shard", [R, N], f32, kind="ExternalOutput").ap()

    a_t = a.rearrange("(t p) n -> t p n", p=P)
    o_t = out.rearrange("(t p) n -> t p n", p=P)

    with tile.TileContext(nc) as tc:
        with (
            tc.tile_pool(name="cpool", bufs=1) as cpool,
            tc.tile_pool(name="spool", bufs=5) as spool,
            tc.tile_pool(name="vpool", bufs=1) as vpool,
            tc.tile_pool(name="psum", bufs=1, space="PSUM") as psum,
            tc.tile_pool(name="dram", bufs=1, space="DRAM") as dram,
        ):
            dsum = vpool.tile([P, T], f32, tag="dsum")
            dinv = vpool.tile([P, T], f32, tag="dinv")
            hpart = vpool.tile([P, NCHUNK * T], f32, tag="hpart")
            cvec = vpool.tile([P, N], f32, tag="cvec")
            ident = vpool.tile([P, P], f32, tag="ident")
            dinv_tp = vpool.tile([T, P], f32, tag="dinv_tp")
            dinv_tpp = psum.tile([T, P], f32, tag="dinv_tpp")
            dloc = dram.tile([1, R], f32, tag="dloc")
            dfull = dram.tile([1, N], f32, tag="dfull")

            masks.make_identity(nc, ident[:, :])

            cached = {}
            # pass 1: row sums; streamed tiles FIRST so their spool slots are
            # free well before the collective (pass-2 prefetch fills the
            # otherwise-dead DMA window); cached tiles in 2MB chunks after.
            # Loads alternate between the Sync and Scalar HWDGE queues to
            # halve per-queue dispatch serialization.
            ld = [nc.sync, nc.scalar]
            nld = 0
            p1_order = [t for t in range(T) if t >= NCACHE] + list(range(NCACHE))
            for t in p1_order:
                nch = NCHUNK
                if t < NCACHE:
                    big = cpool.tile([P, N], f32, tag=f"c{t}")
                    cached[t] = big
                    nch = CCH
                w = N // nch
                for h in range(nch):
                    cols = slice(h * w, (h + 1) * w)
                    if t < NCACHE:
                        tl = cached[t][:, cols]
                    else:
                        stile = spool.tile([P, H], f32, tag="s")
                        tl = stile[:, :]
                    ld[nld % 2].dma_start(out=tl, in_=a_t[t][:, cols])
                    nld += 1
                    c = NCHUNK * t + h
                    nc.vector.reduce_sum(
                        out=hpart[:, c : c + 1], in_=tl, axis=X
                    )
                nc.vector.reduce_sum(
                    out=dsum[:, t : t + 1],
                    in_=hpart[:, NCHUNK * t : NCHUNK * t + nch],
                    axis=X,
                )

            # d^-1/2 (ACT Rsqrt is banned for accuracy; sqrt+reciprocal), then
            # PE-transpose [128, T] -> [T, 128] so the collective input DMA is
            # one contiguous row-ordered 4KB write
            nc.scalar.sqrt(dsum[:, :], dsum[:, :])
            nc.vector.reciprocal(dinv[:, :], dsum[:, :])
            nc.tensor.transpose(dinv_tpp[:, :], dinv[:, :], ident[:, :])
            nc.scalar.copy(dinv_tp[:, :], dinv_tpp[:, :])
            nc.gpsimd.dma_start(out=dloc[0, :], in_=dinv_tp[:, :])

            nc.gpsimd.collective_compute(
                "AllGather",
                mybir.AluOpType.bypass,
                replica_groups=[list(range(NCORES))],
                ins=[dloc[0, :].opt()],
                outs=[dfull[0, :].opt()],
            )

            # replicate the gathered vector across all 128 partitions, chunked
            # so pass-2 chunk c only waits for broadcast chunk c (on the store
            # queue: must NOT block pass-2 prefetch loads on the sync queue)
            for h in range(NCHUNK):
                cols = slice(h * H, (h + 1) * H)
                nc.scalar.dma_start(
                    out=cvec[:, cols],
                    in_=dfull[0:1, cols].to_broadcast((P, H)),
                )

            # pass 2: out = (A * r) * c fused on DVE per chunk; streamed tiles
            # interleaved with cached; end on a streamed tile (its last 1MB
            # store is a shorter tail than a cached tile's 2MB stores)
            un = [t for t in range(T) if t >= NCACHE]
            ca = [t for t in range(T) if t < NCACHE]
            order = [un[0], ca[0], un[1], ca[1], un[2], ca[2], ca[3], un[3]]
            for t in order:
                nch = CCH if t in cached else NCHUNK
                w = N // nch
                for h in range(nch):
                    cols = slice(h * w, (h + 1) * w)
                    if t in cached:
                        tl = cached[t][:, cols]
                    else:
                        stile = spool.tile([P, H], f32, tag="s")
                        tl = stile[:, :]
                        nc.sync.dma_start(out=tl, in_=a_t[t][:, cols])
                    nc.vector.scalar_tensor_tensor(
                        out=tl,
                        in0=tl,
                        scalar=dinv[:, t : t + 1],
                        in1=cvec[:, cols],
                        op0=mult,
                        op1=mult,
                    )
                    nc.scalar.dma_start(out=o_t[t][:, cols], in_=tl)

    nc.compile()
    return nc


def kernel(adjacency_matrix, _trace=False):
    from concourse.bass_utils import run_bass_kernel_spmd

    A = np.ascontiguousarray(np.asarray(adjacency_matrix, dtype=np.float32))
    assert A.shape == (N, N), A.shape

    if "nc" not in _cache:
        _cache["nc"] = _build()
    nc = _cache["nc"]

    in_maps = [{"a_shard": A[c * R : (c + 1) * R]} for c in range(NCORES)]
    res = run_bass_kernel_spmd(
        nc, in_maps, core_ids=list(range(NCORES)), trace=_trace
    )
    _cache["last"] = res
    return np.concatenate(
        [res.results[c]["out_shard"] for c in range(NCORES)], axis=0
    )
